# revision 19
# baseline (speedup 1.0000x reference)
"""DOA pattern loss kernel for Trainium2 (8 NeuronCores, SPMD).

Computes min_r sum_a (possible_phases[r, a] - phases[a])^2 over a
[1_000_000, 32] codebook, returning the scalar min.

Strategy (memory-bound problem):
  - Shard the codebook rows across 8 cores (126976 rows each = 4 * 31744,
    padded with duplicate rows).
  - Host-side, quantize phases to uint8 on a uniform grid over [0, 2pi):
    q = floor(x * 255.5/(2pi) + 0.5), dequantized as x^ = q * SCALE with
    SCALE = 2pi/255.5.  This halves DMA bytes vs fp16 (1 B/elem) and the
    min shifts by only ~1e-3 relative (grid step 0.0246 rad; noise in a
    32-dim squared distance stays well under the 2e-2 gate).
  - Each core's shard [RC, 32] is split into 4 row-quarters and each
    quarter transposed to [32, QPOS]; quarters stack into a [128, QPOS]
    uint8 layout: partition q = 32*quarter + antenna, free dim = row
    position.  The 32-antenna reduction axis lands on SBUF partitions and
    DMA is fully contiguous per partition (8 KB lines).
  - Squares d2 = (SCALE*q - p)^2 in fp16, split across three engines to
    fit under the DMA roofline: ScalarE activation Square(scale*x + bias)
    (u8 in, fused dequant+subtract+square, exact), and VectorE / Pool via
    scalar_tensor_tensor (q*SCALE + (-p)) then tensor_mul.
  - TensorE reduces antenna groups with a tiny stationary ones matrix
    B[128, 32] (B[q, m] = 1 iff q//32 == m//8): out[m, n] = per-row squared
    distance (x8 replicated along m).  Four matmuls per PSUM bank at
    partition offsets 0/32/64/96 pack 8192 distinct row sums per bank.
  - VectorE takes a free-dim min per PSUM bank into a staging column, then
    a final min -> [128, 1] -> DRAM.  Host min over 8 cores x 128 parts.
"""

import numpy as np

P = 128          # SBUF partitions
A = 32           # antennas
NQ = 4           # row-quarters stacked on the partition axis
CHUNK = 512      # matmul rhs free size = one PSUM bank of fp32
NCORES = 8

QPOS = 31744     # row positions per quarter per core = 62 * 512, no ragged tail
RC = NQ * QPOS   # rows per core = 126976
W = 4096         # positions per DMA tile (0.5 MB uint8)

TWO_PI = 2.0 * np.pi
QLEVELS = 255.5  # quantizer: q = floor(x*QLEVELS/2pi + 0.5) in [0, 255]
SCALE = TWO_PI / QLEVELS

_cache: dict = {}


def build_nc(qpos: int = QPOS, w: int = W, reps: int = 1,
             xbufs: int = 4, dbufs: int = 3, tbufs: int = 3, pbufs: int = 6,
             split: tuple = (0.558, 0.320), sq_w: int = 2048):
    """Build the single-core Bass program (same NEFF runs SPMD on all cores).

    reps > 1 repeats the whole compute loop (timing experiments only).
    """
    from contextlib import ExitStack

    import concourse.bacc as bacc
    import concourse.tile as tile
    from concourse import mybir

    dt = mybir.dt.float16
    u8 = mybir.dt.uint8
    nc = bacc.Bacc("TRN2", target_bir_lowering=False)

    cb = nc.dram_tensor("cb", [P, qpos], u8, kind="ExternalInput")
    negp = nc.dram_tensor("negp", [P, 1], mybir.dt.float32, kind="ExternalInput")
    bmat = nc.dram_tensor("bmat", [P, A], dt, kind="ExternalInput")
    out = nc.dram_tensor("out", [P, 1], mybir.dt.float32, kind="ExternalOutput")

    # Free-dim tiling: [offset, width] pairs; widths are chunk multiples.
    # The final w-sized span is split into halving pieces so the pipeline
    # drain (last tile's square -> matmul -> min chain) is short.
    widths = []
    rem = qpos
    while rem > w:
        widths.append(w)
        rem -= w
    while rem > 2 * CHUNK:
        half = ((rem // 2 + CHUNK - 1) // CHUNK) * CHUNK
        widths.append(half)
        rem -= half
    widths.append(rem)
    offs = []
    o = 0
    for wt in widths:
        assert wt % CHUNK == 0 and wt > 0
        offs.append((o, wt))
        o += wt

    n_groups = 0
    for _, wt in offs:
        n_groups += (wt // CHUNK + 3) // 4
    n_groups *= reps

    BIG = 3.0e38  # +inf stand-in (finite, far above any real distance)

    with tile.TileContext(nc) as tc:
        with ExitStack() as ctx:
            singles = ctx.enter_context(tc.tile_pool(name="singles", bufs=1))
            xpool = ctx.enter_context(tc.tile_pool(name="xin", bufs=xbufs))
            dpool = ctx.enter_context(tc.tile_pool(name="d2", bufs=dbufs))
            tpool = ctx.enter_context(tc.tile_pool(name="dtmp", bufs=tbufs))
            ppool = ctx.enter_context(tc.tile_pool(name="ps", bufs=pbufs, space="PSUM"))

            negp_s = singles.tile([P, 1], mybir.dt.float32)
            nc.scalar.dma_start(out=negp_s[:, :], in_=negp[:, :])
            b_s = singles.tile([P, A], dt)
            nc.gpsimd.dma_start(out=b_s[:, :], in_=bmat[:, :])
            stage = singles.tile([P, n_groups], mybir.dt.float32)
            nc.vector.memset(stage[:, :], BIG)
            final = singles.tile([P, 1], mybir.dt.float32)

            # Bresenham accumulators: fraction of square chunks per engine.
            frac_act, frac_pool = split
            acc_a = acc_p = 0.0
            gidx = 0
            for o, wt in offs * reps:
                x = xpool.tile([P, w], u8, tag="x")
                nc.sync.dma_start(out=x[:, :wt], in_=cb[:, o : o + wt])

                d2 = dpool.tile([P, w], dt, tag="d2")
                # Squares emitted per sub-span (finer pipeline deps than the
                # DMA tile), chunks Bresenham-split across ACT / Pool / DVE.
                for so in range(0, wt, sq_w):
                    sw = min(sq_w, wt - so)
                    nch = sw // CHUNK
                    acc_a += nch * frac_act
                    nact = min(int(acc_a + 0.5), nch)
                    acc_a -= nact
                    acc_p += nch * frac_pool
                    npool = min(int(acc_p + 0.5), nch - nact)
                    acc_p -= npool
                    aw = so + nact * CHUNK
                    pw = aw + npool * CHUNK
                    # All paths compute the UNSCALED square (q - p/s)^2; the
                    # s^2 factor is folded into the ones matrix bmat.
                    if aw > so:
                        nc.scalar.activation(
                            d2[:, so:aw],
                            x[:, so:aw],
                            mybir.ActivationFunctionType.Square,
                            bias=negp_s[:, :],
                            scale=1.0,
                        )
                    if pw > aw:
                        nc.gpsimd.tensor_scalar_add(
                            d2[:, aw:pw], x[:, aw:pw], negp_s[:, :]
                        )
                        nc.gpsimd.tensor_mul(
                            d2[:, aw:pw], d2[:, aw:pw], d2[:, aw:pw]
                        )
                    if so + sw > pw:
                        nc.vector.tensor_scalar_add(
                            d2[:, pw : so + sw], x[:, pw : so + sw], negp_s[:, :]
                        )
                        nc.vector.tensor_mul(
                            d2[:, pw : so + sw],
                            d2[:, pw : so + sw],
                            d2[:, pw : so + sw],
                        )

                tile_nch = wt // CHUNK
                for g0 in range(0, tile_nch, 4):
                    gch = min(4, tile_nch - g0)
                    ps = ppool.tile([P, CHUNK], mybir.dt.float32, tag="ps")
                    for jj in range(gch):
                        c = g0 + jj
                        # explicit tile_position: base_partition() rejects 96
                        nc.tensor.matmul(
                            ps[32 * jj : 32 * (jj + 1), :],
                            b_s[:, :],
                            d2[:, c * CHUNK : (c + 1) * CHUNK],
                            start=True,
                            stop=True,
                            tile_position=(0, 32 * jj),
                        )
                    npart = 32 * gch
                    nc.vector.tensor_reduce(
                        out=stage[:npart, gidx : gidx + 1],
                        in_=ps[:npart, :],
                        axis=mybir.AxisListType.X,
                        op=mybir.AluOpType.min,
                    )
                    gidx += 1

            assert gidx == n_groups
            nc.vector.tensor_reduce(
                out=final[:, :],
                in_=stage[:, :],
                axis=mybir.AxisListType.X,
                op=mybir.AluOpType.min,
            )
            nc.sync.dma_start(out=out[:, :], in_=final[:, :])

    nc.compile()
    return nc


def make_in_maps(possible_phases: np.ndarray, phases: np.ndarray, qpos: int = QPOS):
    """Quantize to u8, shard + quarter-transpose; build per-core input maps."""
    rc = NQ * qpos
    rpad = NCORES * rc
    pp = np.asarray(possible_phases, dtype=np.float32)
    q = (pp * (QLEVELS / TWO_PI) + np.float32(0.5)).astype(np.uint8)
    r = q.shape[0]
    assert rpad >= r and rpad - r <= r, (rpad, r)
    if rpad > r:
        # pad with duplicate rows: the min is unchanged
        q = np.concatenate([q, q[: rpad - r]], axis=0)

    ph = np.asarray(phases, dtype=np.float32).reshape(A)
    # Unscaled formulation: bias = -p/s, with s^2 folded into bmat.
    negp = np.tile(-ph / SCALE, NQ).reshape(P, 1).astype(np.float32)
    bmat = np.float16(SCALE * SCALE) * np.kron(
        np.eye(NQ, dtype=np.float16), np.ones((A, A // NQ), dtype=np.float16)
    )  # [128, 32], B[q, m] = s^2 iff q//32 == m//8

    in_maps = []
    for c in range(NCORES):
        shard = q[c * rc : (c + 1) * rc]  # [rc, 32] u8
        cbq = np.ascontiguousarray(
            shard.reshape(NQ, qpos, A).transpose(0, 2, 1).reshape(P, qpos)
        )
        in_maps.append({"cb": cbq, "negp": negp, "bmat": bmat})
    return in_maps


def kernel(possible_phases: np.ndarray, phases: np.ndarray) -> np.ndarray:
    from concourse.bass_utils import run_bass_kernel_spmd

    if "nc" not in _cache:
        _cache["nc"] = build_nc()
    in_maps = make_in_maps(possible_phases, phases)
    res = run_bass_kernel_spmd(_cache["nc"], in_maps, core_ids=list(range(NCORES)))
    mins = np.stack([res.results[c]["out"] for c in range(NCORES)])
    # undo the fp16 rounding of the s^2 factor baked into bmat
    fix = (SCALE * SCALE) / float(np.float16(SCALE * SCALE))
    return np.float32(mins.min() * fix)


# revision 20
# speedup vs baseline: 7.8969x; 7.8969x over previous
"""DOA pattern loss kernel for Trainium2 (8 NeuronCores, SPMD).

Computes min_r sum_a (possible_phases[r, a] - phases[a])^2 over a
[1_000_000, 32] codebook, returning the scalar min.

Strategy (memory-bound problem):
  - Shard the codebook rows across 8 cores (126976 rows each = 4 * 31744,
    padded with duplicate rows).
  - Host-side, quantize phases to uint8 on a uniform grid over [0, 2pi):
    q = floor(x * 255.5/(2pi) + 0.5), dequantized as x^ = q * SCALE with
    SCALE = 2pi/255.5.  This halves DMA bytes vs fp16 (1 B/elem) and the
    min shifts by only ~1e-3 relative (grid step 0.0246 rad; noise in a
    32-dim squared distance stays well under the 2e-2 gate).
  - Each core's shard [RC, 32] is split into 4 row-quarters and each
    quarter transposed to [32, QPOS]; quarters stack into a [128, QPOS]
    uint8 layout: partition q = 32*quarter + antenna, free dim = row
    position.  The 32-antenna reduction axis lands on SBUF partitions and
    DMA is fully contiguous per partition (8 KB lines).
  - Squares d2 = (SCALE*q - p)^2 in fp16, split across three engines to
    fit under the DMA roofline: ScalarE activation Square(scale*x + bias)
    (u8 in, fused dequant+subtract+square, exact), and VectorE / Pool via
    scalar_tensor_tensor (q*SCALE + (-p)) then tensor_mul.
  - TensorE reduces antenna groups with a tiny stationary ones matrix
    B[128, 32] (B[q, m] = 1 iff q//32 == m//8): out[m, n] = per-row squared
    distance (x8 replicated along m).  Four matmuls per PSUM bank at
    partition offsets 0/32/64/96 pack 8192 distinct row sums per bank.
  - VectorE takes a free-dim min per PSUM bank into a staging column, then
    a final min -> [128, 1] -> DRAM.  Host min over 8 cores x 128 parts.
"""

import numpy as np

P = 128          # SBUF partitions
A = 32           # antennas
NQ = 4           # row-quarters stacked on the partition axis
CHUNK = 512      # matmul rhs free size = one PSUM bank of fp32
NCORES = 8

QPOS = 31744     # row positions per quarter per core = 62 * 512, no ragged tail
RC = NQ * QPOS   # rows per core = 126976
W = 4096         # positions per DMA tile (0.5 MB uint8)

TWO_PI = 2.0 * np.pi
QLEVELS = 255.5  # quantizer: q = floor(x*QLEVELS/2pi + 0.5) in [0, 255]
SCALE = TWO_PI / QLEVELS

_cache: dict = {}


def build_nc(qpos: int = QPOS, w: int = W, reps: int = 1,
             xbufs: int = 4, dbufs: int = 3, tbufs: int = 3, pbufs: int = 6,
             split: tuple = (0.73, 0.0), sq_w: int = 2048):
    """Build the single-core Bass program (same NEFF runs SPMD on all cores).

    reps > 1 repeats the whole compute loop (timing experiments only).
    """
    from contextlib import ExitStack

    import concourse.bacc as bacc
    import concourse.tile as tile
    from concourse import mybir

    dt = mybir.dt.float16
    u8 = mybir.dt.uint8
    nc = bacc.Bacc("TRN2", target_bir_lowering=False)

    cb = nc.dram_tensor("cb", [P, qpos], u8, kind="ExternalInput")
    negp = nc.dram_tensor("negp", [P, 1], mybir.dt.float32, kind="ExternalInput")
    bmat = nc.dram_tensor("bmat", [P, A], dt, kind="ExternalInput")
    out = nc.dram_tensor("out", [P, 1], mybir.dt.float32, kind="ExternalOutput")

    # Free-dim tiling: [offset, width] pairs; widths are chunk multiples.
    # The final w-sized span is split into halving pieces so the pipeline
    # drain (last tile's square -> matmul -> min chain) is short.
    widths = []
    rem = qpos
    while rem > w:
        widths.append(w)
        rem -= w
    while rem > 2 * CHUNK:
        half = ((rem // 2 + CHUNK - 1) // CHUNK) * CHUNK
        widths.append(half)
        rem -= half
    widths.append(rem)
    offs = []
    o = 0
    for wt in widths:
        assert wt % CHUNK == 0 and wt > 0
        offs.append((o, wt))
        o += wt

    n_groups = 0
    for _, wt in offs:
        n_groups += (wt // CHUNK + 3) // 4
    n_groups *= reps

    BIG = 3.0e38  # +inf stand-in (finite, far above any real distance)

    with tile.TileContext(nc) as tc:
        with ExitStack() as ctx:
            singles = ctx.enter_context(tc.tile_pool(name="singles", bufs=1))
            xpool = ctx.enter_context(tc.tile_pool(name="xin", bufs=xbufs))
            dpool = ctx.enter_context(tc.tile_pool(name="d2", bufs=dbufs))
            tpool = ctx.enter_context(tc.tile_pool(name="dtmp", bufs=tbufs))
            ppool = ctx.enter_context(tc.tile_pool(name="ps", bufs=pbufs, space="PSUM"))

            negp_s = singles.tile([P, 1], mybir.dt.float32)
            nc.scalar.dma_start(out=negp_s[:, :], in_=negp[:, :])
            b_s = singles.tile([P, A], dt)
            nc.gpsimd.dma_start(out=b_s[:, :], in_=bmat[:, :])
            stage = singles.tile([P, n_groups], mybir.dt.float32)
            nc.vector.memset(stage[:, :], BIG)
            final = singles.tile([P, 1], mybir.dt.float32)

            # Bresenham accumulators: fraction of square chunks per engine.
            frac_act, frac_pool = split
            acc_a = acc_p = 0.0
            gidx = 0
            for o, wt in offs * reps:
                x = xpool.tile([P, w], u8, tag="x")
                nc.sync.dma_start(out=x[:, :wt], in_=cb[:, o : o + wt])

                d2 = dpool.tile([P, w], dt, tag="d2")
                # Squares emitted per sub-span (finer pipeline deps than the
                # DMA tile), chunks Bresenham-split across ACT / Pool / DVE.
                for so in range(0, wt, sq_w):
                    sw = min(sq_w, wt - so)
                    nch = sw // CHUNK
                    acc_a += nch * frac_act
                    nact = min(int(acc_a + 0.5), nch)
                    acc_a -= nact
                    acc_p += nch * frac_pool
                    npool = min(int(acc_p + 0.5), nch - nact)
                    acc_p -= npool
                    aw = so + nact * CHUNK
                    pw = aw + npool * CHUNK
                    # All paths compute the UNSCALED square (q - p/s)^2; the
                    # s^2 factor is folded into the ones matrix bmat.
                    if aw > so:
                        nc.scalar.activation(
                            d2[:, so:aw],
                            x[:, so:aw],
                            mybir.ActivationFunctionType.Square,
                            bias=negp_s[:, :],
                            scale=1.0,
                        )
                    if pw > aw:
                        nc.gpsimd.tensor_scalar_add(
                            d2[:, aw:pw], x[:, aw:pw], negp_s[:, :]
                        )
                        nc.gpsimd.tensor_mul(
                            d2[:, aw:pw], d2[:, aw:pw], d2[:, aw:pw]
                        )
                    if so + sw > pw:
                        nc.vector.tensor_scalar_add(
                            d2[:, pw : so + sw], x[:, pw : so + sw], negp_s[:, :]
                        )
                        nc.vector.tensor_mul(
                            d2[:, pw : so + sw],
                            d2[:, pw : so + sw],
                            d2[:, pw : so + sw],
                        )

                tile_nch = wt // CHUNK
                for g0 in range(0, tile_nch, 4):
                    gch = min(4, tile_nch - g0)
                    ps = ppool.tile([P, CHUNK], mybir.dt.float32, tag="ps")
                    for jj in range(gch):
                        c = g0 + jj
                        # explicit tile_position: base_partition() rejects 96
                        nc.tensor.matmul(
                            ps[32 * jj : 32 * (jj + 1), :],
                            b_s[:, :],
                            d2[:, c * CHUNK : (c + 1) * CHUNK],
                            start=True,
                            stop=True,
                            tile_position=(0, 32 * jj),
                        )
                    npart = 32 * gch
                    nc.vector.tensor_reduce(
                        out=stage[:npart, gidx : gidx + 1],
                        in_=ps[:npart, :],
                        axis=mybir.AxisListType.X,
                        op=mybir.AluOpType.min,
                    )
                    gidx += 1

            assert gidx == n_groups
            nc.vector.tensor_reduce(
                out=final[:, :],
                in_=stage[:, :],
                axis=mybir.AxisListType.X,
                op=mybir.AluOpType.min,
            )
            nc.sync.dma_start(out=out[:, :], in_=final[:, :])

    nc.compile()
    return nc


def make_in_maps(possible_phases: np.ndarray, phases: np.ndarray, qpos: int = QPOS):
    """Quantize to u8, shard + quarter-transpose; build per-core input maps."""
    rc = NQ * qpos
    rpad = NCORES * rc
    pp = np.asarray(possible_phases, dtype=np.float32)
    q = (pp * (QLEVELS / TWO_PI) + np.float32(0.5)).astype(np.uint8)
    r = q.shape[0]
    assert rpad >= r and rpad - r <= r, (rpad, r)
    if rpad > r:
        # pad with duplicate rows: the min is unchanged
        q = np.concatenate([q, q[: rpad - r]], axis=0)

    ph = np.asarray(phases, dtype=np.float32).reshape(A)
    # Unscaled formulation: bias = -p/s, with s^2 folded into bmat.
    negp = np.tile(-ph / SCALE, NQ).reshape(P, 1).astype(np.float32)
    bmat = np.float16(SCALE * SCALE) * np.kron(
        np.eye(NQ, dtype=np.float16), np.ones((A, A // NQ), dtype=np.float16)
    )  # [128, 32], B[q, m] = s^2 iff q//32 == m//8

    in_maps = []
    for c in range(NCORES):
        shard = q[c * rc : (c + 1) * rc]  # [rc, 32] u8
        cbq = np.ascontiguousarray(
            shard.reshape(NQ, qpos, A).transpose(0, 2, 1).reshape(P, qpos)
        )
        in_maps.append({"cb": cbq, "negp": negp, "bmat": bmat})
    return in_maps


def kernel(possible_phases: np.ndarray, phases: np.ndarray) -> np.ndarray:
    from concourse.bass_utils import run_bass_kernel_spmd

    if "nc" not in _cache:
        _cache["nc"] = build_nc()
    in_maps = make_in_maps(possible_phases, phases)
    res = run_bass_kernel_spmd(_cache["nc"], in_maps, core_ids=list(range(NCORES)))
    mins = np.stack([res.results[c]["out"] for c in range(NCORES)])
    # undo the fp16 rounding of the s^2 factor baked into bmat
    fix = (SCALE * SCALE) / float(np.float16(SCALE * SCALE))
    return np.float32(mins.min() * fix)


# revision 21
# speedup vs baseline: 10.1923x; 1.2907x over previous
"""DOA pattern loss kernel for Trainium2 (8 NeuronCores, SPMD).

Computes min_r sum_a (possible_phases[r, a] - phases[a])^2 over a
[1_000_000, 32] codebook, returning the scalar min.

Strategy (memory-bound problem; minimize bytes moved):
  - Quantize the codebook to 4 bits on a uniform grid over [0, 2pi):
    q = round(x * 15.5/(2pi)) in [0, 15], two antennas packed per byte
    -> 16 MB shipped to HBM instead of 128 MB fp32 (8x).
  - Shard rows across 8 cores (126976 rows each = 8 octants x 15872
    positions, padded with duplicate rows).  Per-core layout
    cb4[128, 15872] u8: partition p = 16*octant + antenna-pair, free dim
    = row position, so the antenna reduction lands on SBUF partitions
    and DMA lines are contiguous.
  - Device: DVE extracts nibbles (bitwise and / shift, u8->u8), ScalarE
    squares with the -p/s bias folded into the activation (u8 in, exact),
    TensorE sums antenna groups with two accumulating matmuls per
    512-row chunk (lo + hi nibble planes) against an s^2-scaled ones
    matrix, DVE converts each PSUM bank to u8 distances at 0.5
    granularity (saturating at 255), and a partition-strided DMA writes
    the de-replicated per-row distances out (1 MB total D2H).
  - Host: takes the quantized argmin and exactly rescores (in fp32, from
    the original input) every row whose quantized distance is within
    MARGIN of the quantized min.  The 4-bit pass only has to rank rows
    to within the margin; the returned value is exact (the quantization
    noise std per row is ~1.3, MARGIN = 10 is ~8 sigma).
"""

import numpy as np

P = 128
A = 32
OCT = 8          # row-octants stacked on the partition axis
PAIRS = 16       # antenna pairs (= 2 antennas per byte-column)
CHUNK = 512
NCORES = 8

QP4 = 15872      # positions per octant per core = 31 * 512
RC4 = OCT * QP4  # rows per core = 126976

TWO_PI = 2.0 * np.pi
QL4 = 15.5
S4 = TWO_PI / QL4

OUT_SCALE = 2.0  # u8 distance granularity = 0.5
MARGIN = 10.0    # exactly rescore every row within this of the quantized min

_cache: dict = {}


def build_nc4(qp: int = QP4, w: int = 4096, sq_w: int = 2048, reps: int = 1,
              xbufs: int = 4, dbufs: int = 3, pbufs: int = 6,
              frac_act: float = 1.0):
    from contextlib import ExitStack

    import concourse.bacc as bacc
    import concourse.tile as tile
    from concourse import mybir

    dt = mybir.dt.float16
    u8 = mybir.dt.uint8
    nc = bacc.Bacc("TRN2", target_bir_lowering=False)

    cb = nc.dram_tensor("cb", [P, qp], u8, kind="ExternalInput")
    neglo = nc.dram_tensor("neglo", [P, 1], mybir.dt.float32, kind="ExternalInput")
    neghi = nc.dram_tensor("neghi", [P, 1], mybir.dt.float32, kind="ExternalInput")
    bmat = nc.dram_tensor("bmat", [P, A], dt, kind="ExternalInput")
    nslots = min(reps, 4)  # timing runs reuse output slots round-robin
    outd = nc.dram_tensor("outd", [OCT, qp * nslots], u8, kind="ExternalOutput")

    # DMA tiling with a tapered tail so the pipeline drain is short.
    widths = []
    rem = qp
    while rem > w:
        widths.append(w)
        rem -= w
    while rem > 2 * CHUNK:
        half = ((rem // 2 + CHUNK - 1) // CHUNK) * CHUNK
        widths.append(half)
        rem -= half
    widths.append(rem)
    offs = []
    o = 0
    for wt in widths:
        assert wt % CHUNK == 0 and wt > 0
        offs.append((o, wt))
        o += wt

    with tile.TileContext(nc) as tc:
        with ExitStack() as ctx:
            singles = ctx.enter_context(tc.tile_pool(name="singles", bufs=1))
            xpool = ctx.enter_context(tc.tile_pool(name="xin", bufs=xbufs))
            lpool = ctx.enter_context(tc.tile_pool(name="xl", bufs=dbufs))
            hpool = ctx.enter_context(tc.tile_pool(name="xh", bufs=dbufs))
            dlpool = ctx.enter_context(tc.tile_pool(name="d2l", bufs=dbufs))
            dhpool = ctx.enter_context(tc.tile_pool(name="d2h", bufs=dbufs))
            spool = ctx.enter_context(tc.tile_pool(name="stg", bufs=4))
            ppool = ctx.enter_context(tc.tile_pool(name="ps", bufs=pbufs, space="PSUM"))

            neglo_s = singles.tile([P, 1], mybir.dt.float32)
            nc.scalar.dma_start(out=neglo_s[:, :], in_=neglo[:, :])
            neghi_s = singles.tile([P, 1], mybir.dt.float32)
            nc.scalar.dma_start(out=neghi_s[:, :], in_=neghi[:, :])
            b_s = singles.tile([P, A], dt)
            nc.scalar.dma_start(out=b_s[:, :], in_=bmat[:, :])

            acc_a = 0.0
            for rep in range(reps):
                for o, wt in offs:
                    x = xpool.tile([P, w], u8, tag="x")
                    nc.sync.dma_start(out=x[:, :wt], in_=cb[:, o : o + wt])

                    # bitwise TSP can't cast: extract u8 -> u8, squares cast
                    xl = lpool.tile([P, w], u8, tag="xl")
                    xh = hpool.tile([P, w], u8, tag="xh")
                    d2l = dlpool.tile([P, w], dt, tag="dl")
                    d2h = dhpool.tile([P, w], dt, tag="dh")
                    for so in range(0, wt, sq_w):
                        sw = min(sq_w, wt - so)
                        sl = slice(so, so + sw)
                        nc.vector.tensor_scalar(
                            out=xl[:, sl], in0=x[:, sl],
                            scalar1=15, scalar2=0,
                            op0=mybir.AluOpType.bitwise_and,
                            op1=mybir.AluOpType.bitwise_or,
                        )
                        nc.vector.tensor_scalar(
                            out=xh[:, sl], in0=x[:, sl],
                            scalar1=4, scalar2=0,
                            op0=mybir.AluOpType.logical_shift_right,
                            op1=mybir.AluOpType.bitwise_or,
                        )
                        # squares: ACT with the -p/s bias folded in; a slice
                        # may be moved to DVE by frac_act < 1 (ts_add + mul).
                        nch = sw // CHUNK
                        acc_a += nch * frac_act
                        na = min(int(acc_a + 0.5), nch)
                        acc_a -= na
                        aw = so + na * CHUNK
                        if aw > so:
                            nc.scalar.activation(
                                d2l[:, so:aw], xl[:, so:aw],
                                mybir.ActivationFunctionType.Square,
                                bias=neglo_s[:, :], scale=1.0,
                            )
                            nc.scalar.activation(
                                d2h[:, so:aw], xh[:, so:aw],
                                mybir.ActivationFunctionType.Square,
                                bias=neghi_s[:, :], scale=1.0,
                            )
                        if so + sw > aw:
                            el = slice(aw, so + sw)
                            nc.vector.tensor_scalar_add(
                                d2l[:, el], xl[:, el], neglo_s[:, :]
                            )
                            nc.vector.tensor_mul(
                                d2l[:, el], d2l[:, el], d2l[:, el]
                            )
                            nc.vector.tensor_scalar_add(
                                d2h[:, el], xh[:, el], neghi_s[:, :]
                            )
                            nc.vector.tensor_mul(
                                d2h[:, el], d2h[:, el], d2h[:, el]
                            )

                    tile_nch = wt // CHUNK
                    for g0 in range(0, tile_nch, 4):
                        gch = min(4, tile_nch - g0)
                        ps = ppool.tile([P, CHUNK], mybir.dt.float32, tag="ps")
                        for jj in range(gch):
                            c = g0 + jj
                            cs = slice(c * CHUNK, (c + 1) * CHUNK)
                            nc.tensor.matmul(
                                ps[32 * jj : 32 * (jj + 1), :],
                                b_s[:, :],
                                d2l[:, cs],
                                start=True,
                                stop=False,
                                tile_position=(0, 32 * jj),
                            )
                            nc.tensor.matmul(
                                ps[32 * jj : 32 * (jj + 1), :],
                                b_s[:, :],
                                d2h[:, cs],
                                start=False,
                                stop=True,
                                tile_position=(0, 32 * jj),
                            )
                        npart = 32 * gch
                        stg = spool.tile([P, CHUNK], u8, tag="stg")
                        nc.vector.tensor_scalar(
                            out=stg[:npart, :], in0=ps[:npart, :],
                            scalar1=OUT_SCALE, scalar2=255.0,
                            op0=mybir.AluOpType.mult,
                            op1=mybir.AluOpType.min,
                        )
                        # partitions 0,4,8,... of each 32-block hold the 8
                        # octants' distances exactly once (de-replicated)
                        for jj in range(gch):
                            c = g0 + jj
                            ob = (rep % nslots) * qp
                            nc.sync.dma_start(
                                out=outd[:, ob + o + c * CHUNK
                                         : ob + o + (c + 1) * CHUNK],
                                in_=stg[32 * jj : 32 * (jj + 1) : 4, :],
                            )

    nc.compile()
    return nc


def make_in_maps4(possible_phases: np.ndarray, phases: np.ndarray, qp: int = QP4):
    rc = OCT * qp
    rpad = NCORES * rc
    pp = np.asarray(possible_phases, dtype=np.float32)
    q = np.minimum((pp * (QL4 / TWO_PI) + np.float32(0.5)).astype(np.uint8), 15)
    r = q.shape[0]
    assert rpad >= r and rpad - r <= r, (rpad, r)
    if rpad > r:
        q = np.concatenate([q, q[: rpad - r]], axis=0)  # duplicate-row pad
    packed = (q[:, 0::2] | (q[:, 1::2] << 4)).astype(np.uint8)  # [rpad, 16]

    ph = np.asarray(phases, dtype=np.float32).reshape(A)
    pair = np.arange(P) % PAIRS
    neglo = (-ph[2 * pair] / S4).reshape(P, 1).astype(np.float32)
    neghi = (-ph[2 * pair + 1] / S4).reshape(P, 1).astype(np.float32)
    # B[k, m] = s^2 iff k//16 == m//4 (8 octants, 4x replicated along m)
    bmat = np.float16(S4 * S4) * np.kron(
        np.eye(OCT, dtype=np.float16), np.ones((PAIRS, A // OCT), dtype=np.float16)
    )

    in_maps = []
    for c in range(NCORES):
        shard = packed[c * rc : (c + 1) * rc]  # [rc, 16]
        cbq = np.ascontiguousarray(
            shard.reshape(OCT, qp, PAIRS).transpose(0, 2, 1).reshape(P, qp)
        )
        in_maps.append({"cb": cbq, "neglo": neglo, "neghi": neghi, "bmat": bmat})
    return in_maps


def refine(results, pp: np.ndarray, ph: np.ndarray):
    """Quantized per-row distances -> exact min via host rescore."""
    n_rows = pp.shape[0]
    dist = np.empty(NCORES * RC4, np.float32)
    for c in range(NCORES):
        od = np.asarray(results[c]["outd"])
        dist[c * RC4 : (c + 1) * RC4] = (
            od[:, :QP4].astype(np.float32).reshape(OCT * QP4) / OUT_SCALE
        )
    dmin = dist.min()
    cand = np.nonzero(dist <= dmin + MARGIN)[0]
    cand = cand[cand < n_rows]  # padded rows duplicate row 0.. and stay in set
    if len(cand) == 0:
        cand = np.arange(min(n_rows, 1024))
    diff = pp[cand] - ph
    return np.float32((diff * diff).sum(1).min())


def kernel(possible_phases: np.ndarray, phases: np.ndarray) -> np.ndarray:
    pp = np.asarray(possible_phases, dtype=np.float32)
    ph = np.asarray(phases, dtype=np.float32)
    if pp.shape != (1_000_000, A) or ph.shape != (A,):
        # safety net for unexpected shapes: exact numpy fallback
        diff = pp - ph.reshape(1, -1)
        return np.float32((diff * diff).sum(1).min())

    from concourse.bass_utils import run_bass_kernel_spmd

    if "nc" not in _cache:
        _cache["nc"] = build_nc4()
    in_maps = make_in_maps4(pp, ph)
    res = run_bass_kernel_spmd(_cache["nc"], in_maps, core_ids=list(range(NCORES)))
    return refine(res.results, pp, ph)
